# revision 9
# baseline (speedup 1.0000x reference)
"""Trainium2 Bass kernel for a 2-layer autoregressive LSTM.

Problem: nn_AutoregressiveLSTM (B=512, T=256, I=64, H=512, future_steps=10).
Sharding: pure data parallel — batch is split across 8 NeuronCores, weights
replicated, recurrent state local to each shard.

Per-core layout (BS = 64 batch rows per core):
  - Matmuls run in "M = batch" form: out[batch, gate_chunk] += state.T @ W.T,
    i.e. the (transposed) recurrent state is the PE stationary operand and the
    weight matrix is the moving operand (N = 512 per PSUM bank).  Two
    column-tiled pipes (tile_position (0,0) / (0,64)) run concurrently, one
    producing gate columns for partitions 0:64, the other for 64:128.
  - Gate columns of W are pre-permuted (on host) so that PSUM pair-1 holds
    [i | f] gates and pair-2 holds [g | o] gates, each split into low/high
    hidden halves stacked on the partition axis.  All elementwise work then
    runs as full-width [128, 256] tiles.
  - h/c state lives as [128, 256]: partitions 0:64 <-> hidden 0:256,
    partitions 64:128 <-> hidden 256:512.
  - The per-step state transpose back to stationary form ([hidden, batch])
    is done with two full [128,128] PE transposes per layer.
  - x is pre-transposed AND bf16-cast on the host into [I+1, T, BS] with a
    constant-ones row at partition 64, so the encoder does zero x transposes
    / casts on-device; the layer-0 bias rides the ones row (w0t row 64).
  - The layer-1 bias matmul (K=1) opens each L1 accumulation group so it can
    issue while the PE would otherwise stall on the recurrent-state
    dependency; the x matmuls for step t+1 similarly pad the stall window
    after the h0 transpose.
"""

import numpy as np

import concourse.bass as bass
from concourse import bacc
import concourse.mybir as mybir
import concourse.tile as tile
from concourse.bass_utils import run_bass_kernel_spmd
from concourse.masks import make_identity

F32 = mybir.dt.float32
F32R = mybir.dt.float32r
BF16 = mybir.dt.bfloat16

B, T, I, H = 512, 256, 64, 512
NCORES = 8
BS = B // NCORES  # 64
G = 4 * H  # 2048
NCH = G // 512  # 4 psum chunks per gate set

# Gate-column permutation: new column order is
#   chunk0 = [i_lo, f_lo], chunk1 = [i_hi, f_hi],
#   chunk2 = [g_lo, o_lo], chunk3 = [g_hi, o_hi]
# where lo/hi are hidden halves 0:256 / 256:512 of each 512-wide gate.
GATE_PERM = np.concatenate(
    [
        np.r_[0:256, 512:768],
        np.r_[256:512, 768:1024],
        np.r_[1024:1280, 1536:1792],
        np.r_[1280:1536, 1792:2048],
    ]
)

# hT column order produced by the paired [128,128] transposes: the j-th
# transpose emits K-tile j in cols [128j:128j+64] and K-tile j+2 in
# [128j+64:128j+128].  K-tile k therefore lives at column 64*HT_COL[k].
HT_COL = {0: 0, 1: 2, 2: 1, 3: 3}


def _mm(nc, out, lhsT, rhs, start, stop, tp):
    # skip_group_check: CoreSim's PSUM accumulation-group model is bank-
    # granular, but the hardware tracks has_written per element — two pipes
    # may run independent accumulation groups on disjoint partition halves
    # of one bank (verified on HW: each pipe's start=True clears only its
    # own partition range).
    nc.tensor.matmul(
        out,
        lhsT,
        rhs,
        start=start,
        stop=stop,
        tile_position=tp,
        skip_group_check=True,
    )


def build(T_steps: int, dec_steps: int, mm_mode: str = "bf16"):
    """Emit the Bass module.  Returns nc.

    mm_mode: "bf16" (fast, reduced precision), "w8" (bf16 with the three
    large recurrent weight matrices streamed as fp8-e4m3), or "f32"."""
    MDT = {"bf16": BF16, "f32": F32, "w8": BF16}[mm_mode]
    W8 = mybir.dt.float8e4 if mm_mode == "w8" else MDT
    nc = bacc.Bacc(None, target_bir_lowering=False)

    # x, pre-transposed on host: [I+1, T, BS]; partition 64 = ones.
    x_t = nc.dram_tensor("x_t", [I + 1, T_steps, BS], MDT, kind="ExternalInput")
    w0t = nc.dram_tensor("w0t", [I + 1, G], MDT, kind="ExternalInput")
    wh0 = nc.dram_tensor("wh0", [128, 4, G], W8, kind="ExternalInput")
    w1 = nc.dram_tensor("w1", [128, 8, G], W8, kind="ExternalInput")
    b1r = nc.dram_tensor("b1r", [1, G], MDT, kind="ExternalInput")
    wlin = nc.dram_tensor("wlin", [128, 4, I], MDT, kind="ExternalInput")
    blinr = nc.dram_tensor("blinr", [1, I], MDT, kind="ExternalInput")
    y = nc.dram_tensor("y", [BS, max(dec_steps, 1), I], F32, kind="ExternalOutput")

    CH = 32  # x timesteps per DMA chunk
    Sig = mybir.ActivationFunctionType.Sigmoid
    Tanh = mybir.ActivationFunctionType.Tanh
    Mult = mybir.AluOpType.mult
    Add = mybir.AluOpType.add

    with tile.TileContext(nc) as tc:
        with (
            tc.tile_pool(name="singles", bufs=1) as singles,
            tc.tile_pool(name="xin", bufs=2) as xin_pool,
            tc.tile_pool(name="state", bufs=2) as state_pool,
            tc.tile_pool(name="scratch", bufs=3) as scratch,
            tc.tile_pool(name="pg", bufs=6, space="PSUM") as psum_g,
            tc.tile_pool(name="pt", bufs=1, space="PSUM") as psum_t,
            tc.tile_pool(name="px", bufs=1, space="PSUM") as psum_x,
        ):
            # ---- load weights / constants ----
            w0t_sb = singles.tile([I + 1, G], MDT)
            nc.sync.dma_start(out=w0t_sb, in_=w0t[:, :])
            wh0_sb = singles.tile([128, 4, G], W8)
            nc.sync.dma_start(out=wh0_sb, in_=wh0[:, :, :])
            w1_sb = singles.tile([128, 8, G], W8)
            nc.sync.dma_start(out=w1_sb, in_=w1[:, :, :])
            b1r_sb = singles.tile([1, G], MDT)
            nc.sync.dma_start(out=b1r_sb, in_=b1r[:, :])
            wlin_sb = singles.tile([128, 4, I], MDT)
            nc.sync.dma_start(out=wlin_sb, in_=wlin[:, :, :])
            blinr_sb = singles.tile([1, I], MDT)
            nc.sync.dma_start(out=blinr_sb, in_=blinr[:, :])
            ones_sb = singles.tile([1, BS], MDT)
            nc.vector.memset(ones_sb, 1.0)
            ident = singles.tile([128, 128], F32)
            make_identity(nc, ident)
            if MDT != F32:
                ident_m = singles.tile([128, 128], MDT)
                make_identity(nc, ident_m)
            else:
                ident_m = ident
            out_buf = singles.tile([BS, max(dec_steps, 1), I], F32)

            # ---- persistent state (rotated through pool slots) ----
            h0T = state_pool.tile([128, 256], MDT, tag="h0T")
            h1T = state_pool.tile([128, 256], MDT, tag="h1T")
            c0 = state_pool.tile([128, 256], F32, tag="c0")
            c1 = state_pool.tile([128, 256], F32, tag="c1")
            for t_ in (h0T, h1T, c0, c1):
                nc.vector.memset(t_, 0.0)

            def l0_x_matmuls(x_lhsT):
                """Open layer-0 gate accumulation with the x (+bias row)
                contribution; h0 K-tiles are appended later via
                l0_h_matmuls.  Shared [128,512] banks, one accumulation
                group per partition half."""
                pairs = []
                for pi in range(2):
                    P = psum_g.tile([128, 512], F32, tag="gates")
                    pairs.append(P)
                    for half in range(2):
                        ch = 2 * pi + half
                        outp = P[64 * half : 64 * (half + 1), :]
                        _mm(
                            nc, outp, x_lhsT,
                            w0t_sb[:, 512 * ch : 512 * (ch + 1)],
                            start=True, stop=False, tp=(0, 64 * half),
                        )
                return pairs

            def l0_h_matmuls(pairs, h0T_prev, js=(0, 2, 1, 3), stop=True):
                for pi in range(2):
                    P = pairs[pi]
                    for jj, j in enumerate(js):
                        for half in range(2):
                            ch = 2 * pi + half
                            outp = P[64 * half : 64 * (half + 1), :]
                            lhsT = h0T_prev[:, 64 * HT_COL[j] : 64 * HT_COL[j] + 64]
                            rhs = wh0_sb[:, j, 512 * ch : 512 * (ch + 1)]
                            _mm(
                                nc, outp, lhsT, rhs,
                                start=False,
                                stop=(stop and jj == len(js) - 1),
                                tp=(0, 64 * half),
                            )

            def l1_open_matmuls():
                """Open layer-1 gate accumulation with the bias row (K=1,
                depends on nothing) so the PE has work during recurrent-state
                stalls."""
                pairs = []
                for pi in range(2):
                    P = psum_g.tile([128, 512], F32, tag="gates")
                    pairs.append(P)
                    for half in range(2):
                        ch = 2 * pi + half
                        outp = P[64 * half : 64 * (half + 1), :]
                        _mm(
                            nc, outp, ones_sb,
                            b1r_sb[:, 512 * ch : 512 * (ch + 1)],
                            start=True, stop=False, tp=(0, 64 * half),
                        )
                return pairs

            def l1_h1_matmuls(pairs, h1T_prev):
                """Layer-1 h1-dependent K-tiles (independent of h0(t))."""
                for pi in range(2):
                    P = pairs[pi]
                    for ki in range(4):
                        for half in range(2):
                            ch = 2 * pi + half
                            outp = P[64 * half : 64 * (half + 1), :]
                            lhsT = h1T_prev[:, 64 * HT_COL[ki] : 64 * HT_COL[ki] + 64]
                            rhs = w1_sb[:, 4 + ki, 512 * ch : 512 * (ch + 1)]
                            _mm(
                                nc, outp, lhsT, rhs,
                                start=False, stop=False, tp=(0, 64 * half),
                            )

            def l1_h0_matmuls(pairs, h0T_new):
                for pi in range(2):
                    P = pairs[pi]
                    for jj in range(4):
                        j = (0, 2, 1, 3)[jj]
                        for half in range(2):
                            ch = 2 * pi + half
                            outp = P[64 * half : 64 * (half + 1), :]
                            lhsT = h0T_new[:, 64 * HT_COL[j] : 64 * HT_COL[j] + 64]
                            rhs = w1_sb[:, j, 512 * ch : 512 * (ch + 1)]
                            _mm(
                                nc, outp, lhsT, rhs,
                                start=False,
                                stop=(jj == 3),
                                tp=(0, 64 * half),
                            )

            def elementwise(P1, P2, c_prev, ctag, htag):
                """LSTM cell elementwise on full-width [128, *] tiles.

                Activation outputs are bf16 (bounded values, halves ACT->SBUF
                traffic); the final h mult is all-16-bit so DVE runs it in
                2x_1PORT mode.  c stays fp32 (it accumulates across steps)."""
                S1 = scratch.tile([128, 512], MDT, tag="s1")
                nc.scalar.activation(S1, P1, Sig)
                Tg = scratch.tile([128, 256], MDT, tag="tg")
                nc.scalar.activation(Tg, P2[:, 0:256], Tanh)
                fc = scratch.tile([128, 256], F32, tag="fc")
                nc.vector.tensor_tensor(out=fc, in0=S1[:, 256:512], in1=c_prev, op=Mult)
                tmp = scratch.tile([128, 256], F32, tag="tmp")
                nc.vector.tensor_tensor(out=tmp, in0=S1[:, 0:256], in1=Tg, op=Mult)
                So = scratch.tile([128, 256], MDT, tag="so")
                nc.scalar.activation(So, P2[:, 256:512], Sig)
                c_new = state_pool.tile([128, 256], F32, tag=ctag)
                nc.vector.tensor_tensor(out=c_new, in0=fc, in1=tmp, op=Add)
                Tc = scratch.tile([128, 256], MDT, tag="tc")
                nc.scalar.activation(Tc, c_new, Tanh)
                h_buf = scratch.tile([128, 256], MDT, tag=htag)
                nc.vector.tensor_tensor(out=h_buf, in0=So, in1=Tc, op=Mult)
                return c_new, h_buf

            def transpose_h(h_buf, httag):
                """[128,256] h (batch-major) -> [128,256] hT (hidden-major)."""
                ps = psum_t.tile([128, 256], MDT, tag="tp")
                for j in range(2):
                    nc.tensor.transpose(
                        out=ps[:, 128 * j : 128 * (j + 1)],
                        in_=h_buf[:, 128 * j : 128 * (j + 1)],
                        identity=ident_m,
                    )
                hT_new = state_pool.tile([128, 256], MDT, tag=httag)
                nc.vector.tensor_copy(out=hT_new, in_=ps)
                return hT_new

            # ================= encoder =================
            # Software-pipelined: iteration s completes cell s.  The L1(s+1)
            # group (bias open + h1-dependent K-tiles) is emitted at the END
            # of iteration s so the bias pair and l0_h j=1,3 pairs pad the
            # PE stall while the cell-1 elementwise tail produces h1b(s).
            #   chain0(s) -> h0b; T0 -> h0T(s)
            #   x pair (opens L0(s+1))                <- pads T0/copy0 stall
            #   l1_h0(s) [stops L1(s)]
            #   l0_h(s+1) j=0,2
            #   bias pair (opens L1(s+1))             <- pads T1 stall
            #   l0_h(s+1) j=1,3 [stops L0(s+1)]       <- pads T1 stall
            #   chain1(s) -> h1b; T1 -> h1T(s)
            #   l1_h1(s+1) via h1T(s)
            xt = xin_pool.tile([I + 1, CH, BS], MDT, tag="xc")
            nc.sync.dma_start(out=xt[:, :, :], in_=x_t[:, 0:CH, :])
            cur_slice = xt[:, 0, :]
            P = l0_x_matmuls(cur_slice)
            l0_h_matmuls(P, h0T)
            Q = l1_open_matmuls()
            l1_h1_matmuls(Q, h1T)
            n_cells = T_steps + dec_steps
            for s in range(T_steps):
                nxt = s + 1
                if nxt < T_steps and nxt % CH == 0:
                    nch = min(CH, T_steps - nxt)
                    xt = xin_pool.tile([I + 1, CH, BS], MDT, tag="xc")
                    nc.sync.dma_start(
                        out=xt[:, :nch, :], in_=x_t[:, nxt : nxt + nch, :]
                    )
                if nxt < T_steps:
                    cur_slice = xt[:, nxt % CH, :]
                # else: decode step 0 re-feeds x(T-1) = cur_slice unchanged
                have_next = nxt < T_steps or dec_steps > 0
                c0, h0b = elementwise(P[0], P[1], c0, "c0", "h0b")
                h0T = transpose_h(h0b, "h0T")
                Pn = l0_x_matmuls(cur_slice) if have_next else None
                l1_h0_matmuls(Q, h0T)
                if Pn is not None:
                    l0_h_matmuls(Pn, h0T)
                # K=1 bias pair: near-zero MAC energy, pads the PE stall while
                # the cell-1 elementwise tail produces h1b.
                Qn = l1_open_matmuls() if have_next else None
                c1, h1b = elementwise(Q[0], Q[1], c1, "c1", "h1b")
                h1T = transpose_h(h1b, "h1T")
                if Qn is not None:
                    l1_h1_matmuls(Qn, h1T)
                P, Q = Pn, Qn

            # ================= decoder =================
            for s in range(dec_steps):
                c0, h0b = elementwise(P[0], P[1], c0, "c0", "h0b")
                h0T = transpose_h(h0b, "h0T")
                l1_h0_matmuls(Q, h0T)
                c1, h1b = elementwise(Q[0], Q[1], c1, "c1", "h1b")
                h1T = transpose_h(h1b, "h1T")
                # projection: out[b, i] = h1 @ W_lin.T + b_lin
                po = psum_x.tile([64, I], F32, tag="xt")
                for k in range(5):
                    if k < 4:
                        lhsT = h1T[:, 64 * HT_COL[k] : 64 * HT_COL[k] + 64]
                        rhs = wlin_sb[:, k, :]
                    else:
                        lhsT = ones_sb
                        rhs = blinr_sb
                    _mm(nc, po, lhsT, rhs, k == 0, k == 4, (0, 0))
                nc.vector.tensor_copy(out=out_buf[:, s, :], in_=po)
                if s + 1 < dec_steps:
                    pt = psum_x.tile([64, I], F32, tag="xt")
                    nc.tensor.transpose(
                        out=pt, in_=out_buf[:, s, :], identity=ident[0:64, 0:64]
                    )
                    xdec = scratch.tile([I + 1, 256], MDT, tag="xdec")
                    nc.vector.tensor_copy(out=xdec[0:64, 0:64], in_=pt)
                    nc.vector.memset(xdec[64:65, 0:64], 1.0)
                    P = l0_x_matmuls(xdec[:, 0:64])
                    l0_h_matmuls(P, h0T)
                    Q = l1_open_matmuls()
                    l1_h1_matmuls(Q, h1T)

            nc.sync.dma_start(out=y[:, :, :], in_=out_buf[:, :, :])

    nc.compile()
    return nc


def prep_weights(W_ih0, W_hh0, b_ih0, b_hh0, W_ih1, W_hh1, b_ih1, b_hh1, W_lin, b_lin,
                 mm_mode="bf16"):
    """Host-side packing into the SBUF layouts the kernel expects."""
    import ml_dtypes

    f32 = np.float32
    mdt = np.float32 if mm_mode == "f32" else ml_dtypes.bfloat16
    w8dt = ml_dtypes.float8_e4m3fn if mm_mode == "w8" else mdt
    p = GATE_PERM
    b0 = (np.asarray(b_ih0) + np.asarray(b_hh0)).astype(f32)[p]
    b1 = (np.asarray(b_ih1) + np.asarray(b_hh1)).astype(f32)[p]
    w0t = np.concatenate(
        [np.asarray(W_ih0).T.astype(f32)[:, p], b0[None, :]], axis=0
    )  # [65, G]
    wh0 = (
        np.asarray(W_hh0).T.astype(f32)[:, p].reshape(4, 128, G).transpose(1, 0, 2)
    )  # [128,4,G]
    w1cat = np.concatenate(
        [np.asarray(W_ih1).T.astype(f32), np.asarray(W_hh1).T.astype(f32)], axis=0
    )[:, p]  # [1024, G]
    w1 = w1cat.reshape(8, 128, G).transpose(1, 0, 2)  # [128,8,G]
    wlin = np.asarray(W_lin).T.astype(f32).reshape(4, 128, I).transpose(1, 0, 2)
    return dict(
        w0t=np.ascontiguousarray(w0t.astype(mdt)),
        wh0=np.ascontiguousarray(wh0.astype(w8dt)),
        w1=np.ascontiguousarray(w1.astype(w8dt)),
        b1r=np.ascontiguousarray(b1[None, :].astype(mdt)),
        wlin=np.ascontiguousarray(wlin.astype(mdt)),
        blinr=np.ascontiguousarray(np.asarray(b_lin).astype(f32)[None, :].astype(mdt)),
    )


_cache = {}


def run(x, weights, T_steps, dec_steps, mm_mode="bf16", trace=False):
    """Shard, run on 8 cores, gather.  x: [B, T_steps, I] float32."""
    import ml_dtypes

    key = (T_steps, dec_steps, mm_mode)
    if key not in _cache:
        _cache[key] = build(T_steps, dec_steps, mm_mode)
    nc = _cache[key]
    mdt = np.float32 if mm_mode == "f32" else ml_dtypes.bfloat16
    x = np.asarray(x, dtype=np.float32)
    in_maps = []
    for c in range(NCORES):
        m = dict(weights)
        xs = x[c * BS : (c + 1) * BS]  # [BS, T, I]
        xt = np.empty((I + 1, T_steps, BS), dtype=mdt)
        xt[:I] = xs.transpose(2, 1, 0).astype(mdt)
        xt[I] = mdt(1.0)
        m["x_t"] = np.ascontiguousarray(xt)
        in_maps.append(m)
    res = run_bass_kernel_spmd(nc, in_maps, core_ids=list(range(NCORES)), trace=trace)
    out = np.concatenate([r["y"] for r in res.results], axis=0)
    if dec_steps == 0:
        out = out[:, :0, :]
    return out, res


def kernel(
    x,
    W_ih0,
    W_hh0,
    b_ih0,
    b_hh0,
    W_ih1,
    W_hh1,
    b_ih1,
    b_hh1,
    W_lin,
    b_lin,
    future_steps,
):
    steps = int(future_steps)
    weights = prep_weights(
        W_ih0, W_hh0, b_ih0, b_hh0, W_ih1, W_hh1, b_ih1, b_hh1, W_lin, b_lin,
        mm_mode="bf16",
    )
    x = np.asarray(x, dtype=np.float32)
    out, _ = run(x, weights, x.shape[1], steps, mm_mode="bf16")
    return out
